# revision 30
# baseline (speedup 1.0000x reference)
"""Trainium2 Bass kernel for nn_Classification_4922032521468.

Problem: acts = embeds[activity_index]  (A=512 rows, d=512)
         pairs = concat(acts[ii], acts[jj])  for all i<j (P=130816 pairs)
         out = log_softmax(pairs @ W.T + b)  -> [P, 4]

Key algebra: logits[p, c] = L[i, c] + R'[j, c]  with
  L  = acts @ Wl.T          (Wl = W[:, :512])
  R' = acts @ Wr.T + b      (Wr = W[:, 512:])
so log_softmax needs only lse[i, j] = ln(sum_c e^{L[i,c]} e^{R'[j,c]})
(a K=4 PE matmul of V = e^{R'} against U = e^{L}) and
  out[i, j, c] = L[i, c] + R'[j, c] - lse[i, j].
No 130816x1024 pair tensor is ever built.

vs. the fp32 baseline:
- embeds are staged in HBM as bf16 (tolerance is 2e-2; bf16 lands ~2e-3),
  halving gather bytes and making every PE op single-pass (fp32 is 2-pass).
- FOUR indirect gathers (128 bf16 rows each) land acts chunk-by-chunk
  from ~12us; 16 bf16 PE transposes fill aT [128 d', 4 k, 512 j]; the
  R'/L matmuls (lhsT = wt [128, 37] per d-chunk) run in two column parts
  (chunks 0-2 merged, chunk 3 alone so the final chunk's dependent chain
  is short), producing prm [37, cols]: R'^T on rows 0:4, L^T on rows
  32:36 (compute-engine APs must start on a 32-partition quadrant, so
  lanes 4:8 are out).
- base[j, 4i+c] = L[i, c] + R'[j, c] + b[c] is ONE K=37 matmul per chunk:
  lhsT rows 0:4 = R'^T (bf16 copy of prm), rows 32:36 = ones, row 36 =
  ones; rhs rows 0:4 = E (mask E[c, 4i+c] = 1), rows 32:36 =
  E * L-broadcast (DVE write lane-aligned with prm's L rows), row 36 =
  b tiled 64x (host-built input row - the bias costs zero extra ops).
- se chunk = K=4 matmul of vt x ut4; ut4 = exp(L[:64]) is recomputed by
  a tiny N=64 matmul group so it lands on lanes 0:4 next to vt
  (DVE/ACT lanes are partition-tied; prm's L rows sit on 32:36).
- out chunk = base - ln(se) broadcast: one DVE op per chunk.
- both Exp and Ln live in the 'natural_log_exp_and_others' ACT table set:
  insert_act_table_loads is overridden so walrus emits ONE table load
  (stock pass picks exp_and_others + natural_log = 2 loads with a 1.3us
  stall between the last Exp and the first Ln).

Sharding: core k owns i-rows [64k, 64k+64). Same NEFF on all 8 cores
(SPMD); per-core behavior comes only from per-core DATA: activity_index
is rotated by -64k so each core's own i-rows are gathered rows 0..63.
Each core outputs [512 j, 64 i, 4 c] (j rotated); the host un-rotates j,
transposes, and gathers the triu pairs.
"""

import numpy as np

A = 512  # number of activity tokens
D = 512  # embedding dim
C = 4  # classes
NTOK = 4096  # embeds table rows
RB = 64  # i-rows per core
NCORES = 8
KB = 37  # base-matmul K (R' 0:4, zeros, ones 32:36, bias row 36)

_program = None
_last_results = None  # BassKernelResults from the most recent run (profiling)


def _rtbc_np():
    # rtb rows 4:37 constant half: zeros except rows 32:37 = 1 (the "ones"
    # rows that pair with E*L and the bias row in the base matmul).
    import ml_dtypes

    m = np.zeros((KB - 4, A), dtype=np.float32)
    m[28:33] = 1.0
    return m.astype(ml_dtypes.bfloat16)


def _ident_np():
    import ml_dtypes

    return np.eye(128, dtype=np.float32).astype(ml_dtypes.bfloat16)


def _build_program():
    import types
    from contextlib import ExitStack

    import bass_rust as _bass_rust
    import concourse.bacc as bacc
    import concourse.mybir as mybir
    import concourse.tile as tile
    from concourse.bass import IndirectOffsetOnAxis
    from concourse.hw_specs import get_activation_tables

    fp32 = mybir.dt.float32
    bf16 = mybir.dt.bfloat16
    i32 = mybir.dt.int32
    AF = mybir.ActivationFunctionType
    SUB = mybir.AluOpType.subtract
    MUL = mybir.AluOpType.mult

    nc = bacc.Bacc(
        "TRN2",
        target_bir_lowering=False,
        debug=False,
        enable_asserts=False,
        num_devices=NCORES,
    )

    embt_h = nc.dram_tensor("embt", (NTOK, D), bf16, kind="ExternalInput")
    # idxs[p, j] = rotated activity_index[128j + p], int32
    idx_h = nc.dram_tensor("idxs", (128, 4), i32, kind="ExternalInput")
    # wt[d', 37k+0:4] = Wr.T chunk k, 37k+4:32 = 0, 37k+32:36 = Wl.T chunk k
    wt_h = nc.dram_tensor("wt", (128, 4 * KB), bf16, kind="ExternalInput")
    b4_h = nc.dram_tensor("b4", (C, 1), fp32, kind="ExternalInput")
    # emask rows 0:4 = E (E[c, 4i+c] = 1), 4:32 = 0, 32:36 = E, 36 = b tiled
    emask_h = nc.dram_tensor("emask", (KB, RB * C), mybir.dt.bfloat16,
                             kind="ExternalInput")
    # out[j, 4i + c] (j rotated per core)
    out_h = nc.dram_tensor("out", (A, RB * C), fp32, kind="ExternalOutput")

    rtbc_h = nc.inline_tensor(_rtbc_np(), name="rtbc")
    ident_h = nc.inline_tensor(_ident_np(), name="identb")

    embt_ap = embt_h.ap()
    out_ap = out_h.ap()

    with tile.TileContext(nc) as tc, ExitStack() as ctx:
        sb = ctx.enter_context(tc.tile_pool(name="sb", bufs=1))
        sbr = ctx.enter_context(tc.tile_pool(name="sbr", bufs=4))
        psT = ctx.enter_context(tc.tile_pool(name="psT", bufs=2, space="PSUM"))
        psM = ctx.enter_context(tc.tile_pool(name="psM", bufs=1, space="PSUM"))
        psS = ctx.enter_context(tc.tile_pool(name="psS", bufs=2, space="PSUM"))
        psB = ctx.enter_context(tc.tile_pool(name="psB", bufs=2, space="PSUM"))

        # ---- idx load first, then 4 indirect gathers (128 bf16 rows each) --
        idxt = sb.tile([128, 4], i32, tag="idxt")
        nc.sync.dma_start(out=idxt[:], in_=idx_h.ap()[:])

        actsb = []
        for g in range(4):
            ag = sb.tile([128, D], bf16, tag=f"acts{g}", name=f"acts{g}")
            nc.gpsimd.indirect_dma_start(
                out=ag[:],
                out_offset=None,
                in_=embt_ap[:],
                in_offset=IndirectOffsetOnAxis(ap=idxt[:, g : g + 1], axis=0),
            )
            actsb.append(ag)

        # ---- small constants (scalar queue; overlap the gathers) ----
        identb = sb.tile([128, 128], bf16, tag="identb")
        nc.scalar.dma_start(out=identb[:], in_=ident_h.ap()[:])
        wt = sb.tile([128, 4 * KB], bf16, tag="wt")
        nc.scalar.dma_start(out=wt[:], in_=wt_h.ap()[:])
        b4 = sb.tile([C, 1], fp32, tag="b4")
        nc.scalar.dma_start(out=b4[:], in_=b4_h.ap()[:])
        emask = sb.tile([KB, RB * C], bf16, tag="emask")
        nc.scalar.dma_start(out=emask[:], in_=emask_h.ap()[:])
        erh = sb.tile([KB, RB * C], bf16, tag="erh")  # base-matmul rhs
        nc.scalar.dma_start(out=erh[:], in_=emask_h.ap()[:])
        rtb = sb.tile([KB, A], bf16, tag="rtb")  # rows 0:4 rt, 32:37 ones
        nc.scalar.dma_start(out=rtb[4:KB, :], in_=rtbc_h.ap()[:])

        # ---- transposes fill aT[d', k, j-global]; the R'/L matmuls run
        # in two column parts (chunks 0-2, then chunk 3) so the early tail
        # overlaps chunk-3 work and the final chain stays short. Emission
        # order = engine queue priority: chunk-3 transposes/copies go
        # BEFORE the early tail so the last chunk's copies are not stuck
        # behind the combines on DVE. ----
        aT = sb.tile([128, 4, A], bf16, tag="aT")
        vt = sb.tile([C, A], bf16, tag="vt")  # e^{R'^T + b}
        ut4 = sb.tile([C, RB], bf16, tag="ut4")  # e^{L^T[:, :64]}
        oq = [nc.sync, nc.scalar, nc.sync, nc.scalar]

        def transpose_chunk(j):
            for k in range(4):
                pt = psT.tile([128, 128], bf16, tag="pt", name="pt")
                nc.tensor.transpose(
                    out=pt[:],
                    in_=actsb[j][:, 128 * k : 128 * k + 128],
                    identity=identb[:],
                )
                nc.vector.tensor_copy(
                    out=aT[:, k, 128 * j : 128 * (j + 1)], in_=pt[:]
                )

        def prm_part(c0, c1, pool_tag):
            # prm part: rows 0:4 = R'^T, rows 32:36 = L^T (cols c0:c1)
            if pool_tag == "prm":
                prm = psM.tile([KB, c1 - c0], fp32, tag="prm", name="prmA")
            else:
                # reuse the prl bank (prl is dead once ut4 is computed)
                prm = psS.tile([KB, c1 - c0], fp32, tag="prl", name="prmB",
                               bufs=1)
            for k in range(4):
                nc.tensor.matmul(
                    out=prm[:],
                    lhsT=wt[:, KB * k : KB * (k + 1)],
                    rhs=aT[:, k, c0:c1],
                    start=(k == 0),
                    stop=(k == 3),
                )
            nc.scalar.activation(
                out=vt[:, c0:c1], in_=prm[0:4, :], func=AF.Exp, bias=b4[:]
            )
            for cc in range(c0, c1, 128):
                nc.vector.tensor_copy(
                    out=rtb[0:4, cc : cc + 128], in_=prm[0:4, cc - c0 : cc - c0 + 128]
                )
            if c0 == 0:
                # erh rows 32:36 = E * L bcast (lanes 32:36 = prm L rows)
                nc.vector.tensor_tensor(
                    out=erh[32:36, :].rearrange("p (i c) -> p i c", c=C),
                    in0=emask[32:36, :].rearrange("p (i c) -> p i c", c=C),
                    in1=prm[32:36, 0:RB].unsqueeze(2).to_broadcast([C, RB, C]),
                    op=MUL,
                )

        def tail_chunk(j):
            se = psS.tile([128, RB], fp32, tag="se", name="se")
            nc.tensor.matmul(
                out=se[:],
                lhsT=vt[:, 128 * j : 128 * (j + 1)],
                rhs=ut4[:],
                start=True,
                stop=True,
            )
            lnse = sbr.tile([128, RB], fp32, tag="lnse", name="lnse")
            nc.scalar.activation(out=lnse[:], in_=se[:], func=AF.Ln)

            bs = psB.tile([128, RB * C], fp32, tag="bs", name="bs")
            nc.tensor.matmul(
                out=bs[:],
                lhsT=rtb[:, 128 * j : 128 * (j + 1)],
                rhs=erh[:],
                start=True,
                stop=True,
            )
            oj = sbr.tile([128, RB * C], fp32, tag="oj", name="oj")
            nc.vector.tensor_tensor(
                out=oj[:].rearrange("p (i c) -> p i c", c=C),
                in0=bs[:].rearrange("p (i c) -> p i c", c=C),
                in1=lnse[:].unsqueeze(2).to_broadcast([128, RB, C]),
                op=SUB,
            )
            oq[j].dma_start(out=out_ap[128 * j : 128 * (j + 1), :], in_=oj[:])

        transpose_chunk(0)
        # tiny L-only group so ut4 lands on lanes 0:4 (next to vt)
        prl = psS.tile([C, RB], fp32, tag="prl", name="prl", bufs=1)
        for k in range(4):
            nc.tensor.matmul(
                out=prl[:],
                lhsT=wt[:, KB * k + 32 : KB * k + 36],
                rhs=aT[:, k, 0:RB],
                start=(k == 0),
                stop=(k == 3),
            )
        nc.scalar.activation(out=ut4[:], in_=prl[:], func=AF.Exp)
        transpose_chunk(1)
        transpose_chunk(2)
        prm_part(0, 384, "prm")
        transpose_chunk(3)
        tail_chunk(0)
        tail_chunk(1)
        tail_chunk(2)
        prm_part(384, 512, "prl")
        tail_chunk(3)

    # Both Exp and Ln live in the natural_log_exp_and_others set; doctor
    # the tables list (order/length preserved - index is the set id) so the
    # insertion pass emits ONE load instead of exp_and_others + natural_log.
    def _one_table_load(self):
        has_act = any(
            isinstance(i, mybir.InstActivation)
            for b in self.main_func.blocks
            for i in b.instructions
        )
        if not has_act:
            return
        tables = []
        for name, fns in get_activation_tables(self.m.arch).items():
            if name != "natural_log_exp_and_others":
                fns = fns - {AF.Exp, AF.Ln}
            tables.append((name, fns))
        _bass_rust.insert_act_table_loads(self, tables)

    nc.insert_act_table_loads = types.MethodType(_one_table_load, nc)

    nc.compile()
    return nc


def _get_program():
    global _program
    if _program is None:
        _program = _build_program()
    return _program


def _prep_core_inputs(embt, idx64, wt_np, b4_np, emask_np, k):
    rot = np.roll(idx64, -RB * k)
    idxs = np.ascontiguousarray(rot.reshape(4, 128).T.astype(np.int32))
    return {
        "embt": embt,
        "idxs": idxs,
        "wt": wt_np,
        "b4": b4_np,
        "emask": emask_np,
    }


def kernel(embeds, activity_index, W, b):
    import ml_dtypes
    from concourse.bass_utils import run_bass_kernel_spmd

    bf16 = ml_dtypes.bfloat16
    embt = np.ascontiguousarray(np.asarray(embeds, dtype=np.float32).astype(bf16))
    W = np.asarray(W, dtype=np.float32)
    b_np = np.asarray(b, dtype=np.float32).reshape(C)
    b4_np = np.ascontiguousarray(b_np.reshape(C, 1))
    idx64 = np.asarray(activity_index).astype(np.int64)

    # wt[d, 37k+0:4] = Wr.T chunk k, 37k+4:32 = 0, 37k+32:36 = Wl.T chunk k
    wt_np = np.zeros((128, 4 * KB), dtype=np.float32)
    for k in range(4):
        wt_np[:, KB * k : KB * k + 4] = W[:, D + 128 * k : D + 128 * (k + 1)].T
        wt_np[:, KB * k + 32 : KB * k + 36] = W[:, 128 * k : 128 * (k + 1)].T
    wt_np = np.ascontiguousarray(wt_np.astype(bf16))

    # emask rows 0:4 = E, rows 32:36 = E, row 36 = b tiled 64x
    e = np.zeros((C, RB * C), dtype=np.float32)
    for c in range(C):
        e[c, c::C] = 1.0
    emask_np = np.zeros((KB, RB * C), dtype=np.float32)
    emask_np[0:4] = e
    emask_np[32:36] = e
    emask_np[36] = np.tile(b_np, RB)
    emask_np = np.ascontiguousarray(emask_np.astype(bf16))

    nc = _get_program()
    in_maps = [
        _prep_core_inputs(embt, idx64, wt_np, b4_np, emask_np, k)
        for k in range(NCORES)
    ]

    results = run_bass_kernel_spmd(nc, in_maps, core_ids=list(range(NCORES)))
    global _last_results
    _last_results = results

    out_sq = np.empty((A, A, C), dtype=np.float32)
    for k in range(NCORES):
        # blk[j, i, c] with j rotated by -64k -> un-rotate and transpose
        blk = results.results[k]["out"].reshape(A, RB, C).transpose(1, 0, 2)
        out_sq[RB * k : RB * (k + 1)] = np.roll(blk, RB * k, axis=1)

    ii, jj = np.triu_indices(A, k=1)
    return np.ascontiguousarray(out_sq[ii, jj])


# revision 31
# speedup vs baseline: 1.0039x; 1.0039x over previous
"""Trainium2 Bass kernel for nn_Classification_4922032521468.

Problem: acts = embeds[activity_index]  (A=512 rows, d=512)
         pairs = concat(acts[ii], acts[jj])  for all i<j (P=130816 pairs)
         out = log_softmax(pairs @ W.T + b)  -> [P, 4]

Key algebra: logits[p, c] = L[i, c] + R'[j, c]  with
  L  = acts @ Wl.T          (Wl = W[:, :512])
  R' = acts @ Wr.T + b      (Wr = W[:, 512:])
so log_softmax needs only lse[i, j] = ln(sum_c e^{L[i,c]} e^{R'[j,c]})
(a K=4 PE matmul of V = e^{R'} against U = e^{L}) and
  out[i, j, c] = L[i, c] + R'[j, c] - lse[i, j].
No 130816x1024 pair tensor is ever built.

vs. the fp32 baseline:
- embeds are staged in HBM as bf16 (tolerance is 2e-2; bf16 lands ~2e-3),
  halving gather bytes and making every PE op single-pass (fp32 is 2-pass).
- FOUR indirect gathers (128 bf16 rows each) land acts chunk-by-chunk
  from ~12us; 16 bf16 PE transposes fill aT [128 d', 4 k, 512 j]; the
  R'/L matmuls (lhsT = wt [128, 37] per d-chunk) run in two column parts
  (chunks 0-2 merged, chunk 3 alone so the final chunk's dependent chain
  is short), producing prm [37, cols]: R'^T on rows 0:4, L^T on rows
  32:36 (compute-engine APs must start on a 32-partition quadrant, so
  lanes 4:8 are out).
- base[j, 4i+c] = L[i, c] + R'[j, c] + b[c] is ONE K=37 matmul per chunk:
  lhsT rows 0:4 = R'^T (bf16 copy of prm), rows 32:36 = ones, row 36 =
  ones; rhs rows 0:4 = E (mask E[c, 4i+c] = 1), rows 32:36 =
  E * L-broadcast (DVE write lane-aligned with prm's L rows), row 36 =
  b tiled 64x (host-built input row - the bias costs zero extra ops).
- se chunk = K=4 matmul of vt x ut4; ut4 = exp(L[:64]) is recomputed by
  a tiny N=64 matmul group so it lands on lanes 0:4 next to vt
  (DVE/ACT lanes are partition-tied; prm's L rows sit on 32:36).
- out chunk = base - ln(se) broadcast: one DVE op per chunk.
- both Exp and Ln live in the 'natural_log_exp_and_others' ACT table set:
  insert_act_table_loads is overridden so walrus emits ONE table load
  (stock pass picks exp_and_others + natural_log = 2 loads with a 1.3us
  stall between the last Exp and the first Ln).

Sharding: core k owns i-rows [64k, 64k+64). Same NEFF on all 8 cores
(SPMD); per-core behavior comes only from per-core DATA: activity_index
is rotated by -64k so each core's own i-rows are gathered rows 0..63.
Each core outputs [512 j, 64 i, 4 c] (j rotated); the host un-rotates j,
transposes, and gathers the triu pairs.
"""

import numpy as np

A = 512  # number of activity tokens
D = 512  # embedding dim
C = 4  # classes
NTOK = 4096  # embeds table rows
RB = 64  # i-rows per core
NCORES = 8
KB = 37  # base-matmul K (R' 0:4, zeros, ones 32:36, bias row 36)

_program = None
_last_results = None  # BassKernelResults from the most recent run (profiling)


def _rtbc_np():
    # rtb rows 4:37 constant half: zeros except rows 32:37 = 1 (the "ones"
    # rows that pair with E*L and the bias row in the base matmul).
    import ml_dtypes

    m = np.zeros((KB - 4, A), dtype=np.float32)
    m[28:33] = 1.0
    return m.astype(ml_dtypes.bfloat16)


def _ident_np():
    import ml_dtypes

    return np.eye(128, dtype=np.float32).astype(ml_dtypes.bfloat16)


def _build_program():
    import types
    from contextlib import ExitStack

    import bass_rust as _bass_rust
    import concourse.bacc as bacc
    import concourse.mybir as mybir
    import concourse.tile as tile
    from concourse.bass import IndirectOffsetOnAxis
    from concourse.hw_specs import get_activation_tables

    fp32 = mybir.dt.float32
    bf16 = mybir.dt.bfloat16
    i32 = mybir.dt.int32
    AF = mybir.ActivationFunctionType
    SUB = mybir.AluOpType.subtract
    MUL = mybir.AluOpType.mult

    nc = bacc.Bacc(
        "TRN2",
        target_bir_lowering=False,
        debug=False,
        enable_asserts=False,
        num_devices=NCORES,
    )

    embt_h = nc.dram_tensor("embt", (NTOK, D), bf16, kind="ExternalInput")
    # idxs[p, j] = rotated activity_index[128j + p], int32
    idx_h = nc.dram_tensor("idxs", (128, 4), i32, kind="ExternalInput")
    # wt[d', 37k+0:4] = Wr.T chunk k, 37k+4:32 = 0, 37k+32:36 = Wl.T chunk k
    wt_h = nc.dram_tensor("wt", (128, 4 * KB), bf16, kind="ExternalInput")
    b4_h = nc.dram_tensor("b4", (C, 1), fp32, kind="ExternalInput")
    # emask rows 0:4 = E (E[c, 4i+c] = 1), 4:32 = 0, 32:36 = E, 36 = b tiled
    emask_h = nc.dram_tensor("emask", (KB, RB * C), mybir.dt.bfloat16,
                             kind="ExternalInput")
    # out[j, 4i + c] (j rotated per core)
    out_h = nc.dram_tensor("out", (A, RB * C), fp32, kind="ExternalOutput")

    rtbc_h = nc.inline_tensor(_rtbc_np(), name="rtbc")
    ident_h = nc.inline_tensor(_ident_np(), name="identb")

    embt_ap = embt_h.ap()
    out_ap = out_h.ap()

    with tile.TileContext(nc) as tc, ExitStack() as ctx:
        sb = ctx.enter_context(tc.tile_pool(name="sb", bufs=1))
        sbr = ctx.enter_context(tc.tile_pool(name="sbr", bufs=4))
        psT = ctx.enter_context(tc.tile_pool(name="psT", bufs=2, space="PSUM"))
        psM = ctx.enter_context(tc.tile_pool(name="psM", bufs=1, space="PSUM"))
        psS = ctx.enter_context(tc.tile_pool(name="psS", bufs=2, space="PSUM"))
        psB = ctx.enter_context(tc.tile_pool(name="psB", bufs=2, space="PSUM"))

        # ---- idx load first, then 4 indirect gathers (128 bf16 rows each) --
        idxt = sb.tile([128, 4], i32, tag="idxt")
        nc.sync.dma_start(out=idxt[:], in_=idx_h.ap()[:])

        actsb = []
        for g in range(4):
            ag = sb.tile([128, D], bf16, tag=f"acts{g}", name=f"acts{g}")
            nc.gpsimd.indirect_dma_start(
                out=ag[:],
                out_offset=None,
                in_=embt_ap[:],
                in_offset=IndirectOffsetOnAxis(ap=idxt[:, g : g + 1], axis=0),
            )
            actsb.append(ag)

        # ---- small constants (scalar queue; overlap the gathers) ----
        identb = sb.tile([128, 128], bf16, tag="identb")
        nc.scalar.dma_start(out=identb[:], in_=ident_h.ap()[:])
        wt = sb.tile([128, 4 * KB], bf16, tag="wt")
        nc.scalar.dma_start(out=wt[:], in_=wt_h.ap()[:])
        b4 = sb.tile([C, 1], fp32, tag="b4")
        nc.scalar.dma_start(out=b4[:], in_=b4_h.ap()[:])
        emask = sb.tile([KB, RB * C], bf16, tag="emask")
        nc.scalar.dma_start(out=emask[:], in_=emask_h.ap()[:])
        erh = sb.tile([KB, RB * C], bf16, tag="erh")  # base-matmul rhs
        nc.scalar.dma_start(out=erh[:], in_=emask_h.ap()[:])
        rtb = sb.tile([KB, A], bf16, tag="rtb")  # rows 0:4 rt, 32:37 ones
        nc.scalar.dma_start(out=rtb[4:KB, :], in_=rtbc_h.ap()[:])

        # ---- transposes fill aT[d', k, j-global]; the R'/L matmuls run
        # in two column parts (chunks 0-2, then chunk 3) so the early tail
        # overlaps chunk-3 work and the final chain stays short. Emission
        # order = engine queue priority: chunk-3 transposes/copies go
        # BEFORE the early tail so the last chunk's copies are not stuck
        # behind the combines on DVE. ----
        aT = sb.tile([128, 4, A], bf16, tag="aT")
        vt = sb.tile([C, A], bf16, tag="vt")  # e^{R'^T + b}
        ut4 = sb.tile([C, RB], bf16, tag="ut4")  # e^{L^T[:, :64]}
        oq = [nc.sync, nc.scalar, nc.sync, nc.scalar]

        def transpose_chunk(j):
            for k in range(4):
                pt = psT.tile([128, 128], bf16, tag="pt", name="pt")
                nc.tensor.transpose(
                    out=pt[:],
                    in_=actsb[j][:, 128 * k : 128 * k + 128],
                    identity=identb[:],
                )
                nc.vector.tensor_copy(
                    out=aT[:, k, 128 * j : 128 * (j + 1)], in_=pt[:]
                )

        def prm_part(c0, c1, pool_tag):
            # prm part: rows 0:4 = R'^T, rows 32:36 = L^T (cols c0:c1)
            if pool_tag == "prm":
                prm = psM.tile([KB, c1 - c0], fp32, tag="prm", name="prmA")
            else:
                # reuse the prl bank (prl is dead once ut4 is computed)
                prm = psS.tile([KB, c1 - c0], fp32, tag="prl", name="prmB",
                               bufs=1)
            for k in range(4):
                nc.tensor.matmul(
                    out=prm[:],
                    lhsT=wt[:, KB * k : KB * (k + 1)],
                    rhs=aT[:, k, c0:c1],
                    start=(k == 0),
                    stop=(k == 3),
                )
            nc.scalar.activation(
                out=vt[:, c0:c1], in_=prm[0:4, :], func=AF.Exp, bias=b4[:]
            )
            if c0 == 0:
                # erh rows 32:36 = E * L bcast (lanes 32:36 = prm L rows);
                # FIRST on DVE - it gates every base matmul
                nc.vector.tensor_tensor(
                    out=erh[32:36, :].rearrange("p (i c) -> p i c", c=C),
                    in0=emask[32:36, :].rearrange("p (i c) -> p i c", c=C),
                    in1=prm[32:36, 0:RB].unsqueeze(2).to_broadcast([C, RB, C]),
                    op=MUL,
                )
            for cc in range(c0, c1, 128):
                nc.vector.tensor_copy(
                    out=rtb[0:4, cc : cc + 128], in_=prm[0:4, cc - c0 : cc - c0 + 128]
                )

        def tail_chunk(j):
            se = psS.tile([128, RB], fp32, tag="se", name="se")
            nc.tensor.matmul(
                out=se[:],
                lhsT=vt[:, 128 * j : 128 * (j + 1)],
                rhs=ut4[:],
                start=True,
                stop=True,
            )
            lnse = sbr.tile([128, RB], fp32, tag="lnse", name="lnse")
            nc.scalar.activation(out=lnse[:], in_=se[:], func=AF.Ln)

            bs = psB.tile([128, RB * C], fp32, tag="bs", name="bs")
            nc.tensor.matmul(
                out=bs[:],
                lhsT=rtb[:, 128 * j : 128 * (j + 1)],
                rhs=erh[:],
                start=True,
                stop=True,
            )
            oj = sbr.tile([128, RB * C], fp32, tag="oj", name="oj")
            nc.vector.tensor_tensor(
                out=oj[:].rearrange("p (i c) -> p i c", c=C),
                in0=bs[:].rearrange("p (i c) -> p i c", c=C),
                in1=lnse[:].unsqueeze(2).to_broadcast([128, RB, C]),
                op=SUB,
            )
            oq[j].dma_start(out=out_ap[128 * j : 128 * (j + 1), :], in_=oj[:])

        transpose_chunk(0)
        # tiny L-only group so ut4 lands on lanes 0:4 (next to vt)
        prl = psS.tile([C, RB], fp32, tag="prl", name="prl", bufs=1)
        for k in range(4):
            nc.tensor.matmul(
                out=prl[:],
                lhsT=wt[:, KB * k + 32 : KB * k + 36],
                rhs=aT[:, k, 0:RB],
                start=(k == 0),
                stop=(k == 3),
            )
        nc.scalar.activation(out=ut4[:], in_=prl[:], func=AF.Exp)
        transpose_chunk(1)
        transpose_chunk(2)
        prm_part(0, 384, "prm")
        transpose_chunk(3)
        tail_chunk(0)
        tail_chunk(1)
        tail_chunk(2)
        prm_part(384, 512, "prl")
        tail_chunk(3)

    # Both Exp and Ln live in the natural_log_exp_and_others set; doctor
    # the tables list (order/length preserved - index is the set id) so the
    # insertion pass emits ONE load instead of exp_and_others + natural_log.
    def _one_table_load(self):
        has_act = any(
            isinstance(i, mybir.InstActivation)
            for b in self.main_func.blocks
            for i in b.instructions
        )
        if not has_act:
            return
        tables = []
        for name, fns in get_activation_tables(self.m.arch).items():
            if name != "natural_log_exp_and_others":
                fns = fns - {AF.Exp, AF.Ln}
            tables.append((name, fns))
        _bass_rust.insert_act_table_loads(self, tables)

    nc.insert_act_table_loads = types.MethodType(_one_table_load, nc)

    nc.compile()
    return nc


def _get_program():
    global _program
    if _program is None:
        _program = _build_program()
    return _program


def _prep_core_inputs(embt, idx64, wt_np, b4_np, emask_np, k):
    rot = np.roll(idx64, -RB * k)
    idxs = np.ascontiguousarray(rot.reshape(4, 128).T.astype(np.int32))
    return {
        "embt": embt,
        "idxs": idxs,
        "wt": wt_np,
        "b4": b4_np,
        "emask": emask_np,
    }


def kernel(embeds, activity_index, W, b):
    import ml_dtypes
    from concourse.bass_utils import run_bass_kernel_spmd

    bf16 = ml_dtypes.bfloat16
    embt = np.ascontiguousarray(np.asarray(embeds, dtype=np.float32).astype(bf16))
    W = np.asarray(W, dtype=np.float32)
    b_np = np.asarray(b, dtype=np.float32).reshape(C)
    b4_np = np.ascontiguousarray(b_np.reshape(C, 1))
    idx64 = np.asarray(activity_index).astype(np.int64)

    # wt[d, 37k+0:4] = Wr.T chunk k, 37k+4:32 = 0, 37k+32:36 = Wl.T chunk k
    wt_np = np.zeros((128, 4 * KB), dtype=np.float32)
    for k in range(4):
        wt_np[:, KB * k : KB * k + 4] = W[:, D + 128 * k : D + 128 * (k + 1)].T
        wt_np[:, KB * k + 32 : KB * k + 36] = W[:, 128 * k : 128 * (k + 1)].T
    wt_np = np.ascontiguousarray(wt_np.astype(bf16))

    # emask rows 0:4 = E, rows 32:36 = E, row 36 = b tiled 64x
    e = np.zeros((C, RB * C), dtype=np.float32)
    for c in range(C):
        e[c, c::C] = 1.0
    emask_np = np.zeros((KB, RB * C), dtype=np.float32)
    emask_np[0:4] = e
    emask_np[32:36] = e
    emask_np[36] = np.tile(b_np, RB)
    emask_np = np.ascontiguousarray(emask_np.astype(bf16))

    nc = _get_program()
    in_maps = [
        _prep_core_inputs(embt, idx64, wt_np, b4_np, emask_np, k)
        for k in range(NCORES)
    ]

    results = run_bass_kernel_spmd(nc, in_maps, core_ids=list(range(NCORES)))
    global _last_results
    _last_results = results

    out_sq = np.empty((A, A, C), dtype=np.float32)
    for k in range(NCORES):
        # blk[j, i, c] with j rotated by -64k -> un-rotate and transpose
        blk = results.results[k]["out"].reshape(A, RB, C).transpose(1, 0, 2)
        out_sq[RB * k : RB * (k + 1)] = np.roll(blk, RB * k, axis=1)

    ii, jj = np.triu_indices(A, k=1)
    return np.ascontiguousarray(out_sq[ii, jj])
